# revision 2
# baseline (speedup 1.0000x reference)
"""Trainium2 kernel for nn_ClsSegLoss (cls BCE + masked dice seg loss).

Strategy (data-parallel over batch, 8 NeuronCores):
  - The only heavy data is predict_seg and masks (64 MiB each).  Each core
    streams its 8 samples (2 MiB/sample) and computes four per-sample
    reductions:
        psum = sum(sigmoid(seg))      [ACT sigmoid pass, free accum_out]
        gg   = sum(mask)  (== sum(mask^2), mask is binary)
                                      [ACT copy pass, free accum_out]
        pg   = sum(sigmoid(seg)*mask) [DVE scalar_tensor_tensor + accum_out]
        pp   = sum(sigmoid(seg)^2)    [DVE scalar_tensor_tensor + accum_out]
    Each op leaves per-partition partial sums in a [128, 1] column; the
    [128, 16] accumulator tiles are DMA'd out and the final 128-way sums +
    dice/BCE scalar math (O(64) flops) run on host in float64.
  - DMA-bound by design: 16 MiB HBM reads/core ~ 47 us at ~358 GB/s; ACT
    (~2 passes/sample ~ 32 us) and DVE (~2 passes/sample ~ 37 us) hide
    underneath.
  - This walrus build rejects instructions carrying more than one sync
    wait ("Too many sync wait commands"), while Tile freely attaches
    several (esp. the TileContext-exit drain).  _split_excess_waits() moves
    surplus waits onto same-engine NoOps inserted just before the
    instruction, which is semantically identical on in-order sequencers.
"""

import sys

import numpy as np

for _p in ("/opt/trn_rl_repo",):
    if _p not in sys.path:
        sys.path.insert(0, _p)

import concourse.bass as bass
import concourse.tile as tile
from concourse import mybir
from concourse.bass_utils import run_bass_kernel_spmd

B, C, H, W = 64, 1, 512, 512
N_CORES = 8
SPC = B // N_CORES  # samples per core = 8
N = C * H * W  # elements per sample = 262144
P = 128  # SBUF partitions
F = N // P  # free dim per sample tile = 2048

_F32 = mybir.dt.float32

_split_ctr = [0]


def _split_excess_waits(nc: bass.Bass, max_waits: int = 1) -> bass.Bass:
    """Move surplus sync waits onto same-engine NoOps (walrus allows only
    one wait per instruction in this build)."""
    for bb in nc.main_func.blocks:
        insts = bb.instructions
        new = []
        changed = False
        for ins in insts:
            si = getattr(ins, "sync_info", None)
            waits = list(si.on_wait) if (si is not None and si.on_wait) else []
            if len(waits) > max_waits:
                keep = waits[-max_waits:]
                extra = waits[:-max_waits]
                for k in range(0, len(extra), max_waits):
                    chunk = extra[k : k + max_waits]
                    _split_ctr[0] += 1
                    new.append(
                        mybir.InstNoOp(
                            name=f"ant_wait_split_{_split_ctr[0]}",
                            engine=ins.engine,
                            ins=[],
                            outs=[],
                            sync_info=mybir.SyncInfo(on_wait=chunk, on_update=[]),
                        )
                    )
                ins.sync_info = mybir.SyncInfo(
                    on_wait=keep, on_update=list(si.on_update)
                )
                changed = True
            new.append(ins)
        if changed:
            insts[:] = new
    return nc


def _build_nc() -> bass.Bass:
    nc = bass.Bass()
    seg = nc.declare_dram_parameter("seg", [SPC, P, F], _F32, isOutput=False)
    msk = nc.declare_dram_parameter("msk", [SPC, P, F], _F32, isOutput=False)
    # res_act[:, i] = per-partition sum(sigmoid(seg_i)); res_act[:, SPC+i] = sum(msk_i)
    # res_dve[:, i] = per-partition sum(sig_i*msk_i);    res_dve[:, SPC+i] = sum(sig_i^2)
    res_act = nc.declare_dram_parameter("res_act", [P, 2 * SPC], _F32, isOutput=True)
    res_dve = nc.declare_dram_parameter("res_dve", [P, 2 * SPC], _F32, isOutput=True)

    seg_ap = seg[:]
    msk_ap = msk[:]

    AF = mybir.ActivationFunctionType
    OP = mybir.AluOpType

    with tile.TileContext(nc) as tc:
        with (
            tc.tile_pool(name="seg_p", bufs=3) as seg_p,
            tc.tile_pool(name="msk_p", bufs=3) as msk_p,
            tc.tile_pool(name="sig_p", bufs=3) as sig_p,
            tc.tile_pool(name="dmp_a", bufs=2) as dmp_a,
            tc.tile_pool(name="dmp_v", bufs=2) as dmp_v,
            tc.tile_pool(name="acc_p", bufs=1) as acc_p,
        ):
            acc_a = acc_p.tile([P, 2 * SPC], _F32)
            acc_v = acc_p.tile([P, 2 * SPC], _F32)
            for i in range(SPC):
                s = seg_p.tile([P, F], _F32, tag="s")
                m = msk_p.tile([P, F], _F32, tag="m")
                nc.sync.dma_start(out=s, in_=seg_ap[i])
                nc.sync.dma_start(out=m, in_=msk_ap[i])

                g = sig_p.tile([P, F], _F32, tag="g")
                da = dmp_a.tile([P, F], _F32, tag="da")
                # g = sigmoid(s); acc_a[:, i] = rowsum(g)
                nc.scalar.activation(g, s, AF.Sigmoid, accum_out=acc_a[:, i : i + 1])
                # da = m; acc_a[:, SPC+i] = rowsum(m)
                nc.scalar.activation(
                    da, m, AF.Copy, accum_out=acc_a[:, SPC + i : SPC + i + 1]
                )

                dv0 = dmp_v.tile([P, F], _F32, tag="dv0")
                dv1 = dmp_v.tile([P, F], _F32, tag="dv1")
                # dv0 = (g*1)*m; acc_v[:, i] = rowsum(g*m)
                nc.vector.scalar_tensor_tensor(
                    out=dv0,
                    in0=g,
                    scalar=1.0,
                    in1=m,
                    op0=OP.mult,
                    op1=OP.mult,
                    accum_out=acc_v[:, i : i + 1],
                )
                # dv1 = (g*1)*g; acc_v[:, SPC+i] = rowsum(g*g)
                nc.vector.scalar_tensor_tensor(
                    out=dv1,
                    in0=g,
                    scalar=1.0,
                    in1=g,
                    op0=OP.mult,
                    op1=OP.mult,
                    accum_out=acc_v[:, SPC + i : SPC + i + 1],
                )
            nc.sync.dma_start(out=res_act[:], in_=acc_a)
            nc.sync.dma_start(out=res_dve[:], in_=acc_v)
    return _split_excess_waits(nc)


_NC_CACHE: bass.Bass | None = None


def _get_nc() -> bass.Bass:
    global _NC_CACHE
    if _NC_CACHE is None:
        _NC_CACHE = _build_nc()
    return _NC_CACHE


def _device_sums(seg: np.ndarray, msk: np.ndarray, **spmd_kwargs):
    """seg, msk: [B, N] float32.  Returns (pg, pp, gg, psum) each [B] float64,
    plus the raw BassKernelResults (for profiling)."""
    in_maps = []
    for c in range(N_CORES):
        sl = slice(c * SPC, (c + 1) * SPC)
        in_maps.append(
            {
                "seg": np.ascontiguousarray(seg[sl].reshape(SPC, P, F)),
                "msk": np.ascontiguousarray(msk[sl].reshape(SPC, P, F)),
            }
        )
    out = run_bass_kernel_spmd(_get_nc(), in_maps, list(range(N_CORES)), **spmd_kwargs)
    pg = np.empty(B, np.float64)
    pp = np.empty(B, np.float64)
    gg = np.empty(B, np.float64)
    psum = np.empty(B, np.float64)
    for c in range(N_CORES):
        sl = slice(c * SPC, (c + 1) * SPC)
        ra = np.asarray(out.results[c]["res_act"], dtype=np.float64)
        rv = np.asarray(out.results[c]["res_dve"], dtype=np.float64)
        psum[sl] = ra[:, :SPC].sum(axis=0)
        gg[sl] = ra[:, SPC:].sum(axis=0)
        pg[sl] = rv[:, :SPC].sum(axis=0)
        pp[sl] = rv[:, SPC:].sum(axis=0)
    return pg, pp, gg, psum, out


def kernel(predict_cls, predict_seg, labels, masks):
    seg = np.asarray(predict_seg, dtype=np.float32).reshape(B, N)
    msk = np.asarray(masks, dtype=np.float32).reshape(B, N)
    pg, pp, gg, psum, _ = _device_sums(seg, msk)

    pc = np.asarray(predict_cls, dtype=np.float64)
    lab = np.asarray(labels).astype(np.float64)

    # classification BCE (mean reduction)
    eps = 1e-7
    pc_c = np.clip(pc, eps, 1.0 - eps)
    cls_loss = -np.mean(lab * np.log(pc_c) + (1.0 - lab) * np.log(1.0 - pc_c))

    # dice over samples selected by predict_cls >= 0.5
    dice_pos = (2.0 * pg + 1e-5) / (pp + gg + 1e-5)
    dice_neg = 25.0 / (psum + 25.0)
    dice = np.where(lab == 1.0, dice_pos, dice_neg)
    sel = (pc >= 0.5).astype(np.float64)
    n = sel.sum()
    dice_sum = (sel * dice).sum()
    if n > 0.0:
        seg_loss = (n - dice_sum) / max(n, 1.0)
    else:
        seg_loss = 1e-4
    return (np.float32(cls_loss), np.float32(seg_loss))


# revision 3
# speedup vs baseline: 1.7682x; 1.7682x over previous
"""Trainium2 kernel for nn_ClsSegLoss (cls BCE + masked dice seg loss).

Strategy (data-parallel over batch, 8 NeuronCores):
  - cls BCE needs only predict_cls/labels (64 floats) -> host.
  - seg dice only involves samples with predict_cls >= 0.5 ("selected"):
      label==1 selected samples need pg=sum(sig*m), pp=sum(sig^2),
               gg=sum(m)  (mask binary -> == sum(m^2))
      label!=1 selected samples need only psum=sum(sig)
      unselected samples contribute nothing.
    predict_cls/labels are host-visible, so kernel() builds the exact work
    list per call and ships only the needed samples, fp16-cast (masks are
    exactly {0,1} -> fp16 lossless; fp16 logits add ~1e-5 rel err to the
    262144-element sums).
  - Per core: K1 "full" samples (seg+mask, ACT sigmoid -> DVE
    scalar_tensor_tensor pg/pp + tensor_scalar gg, all with fused fp32
    accum_out) and K0 "sig-only" samples (ACT sigmoid with accum_out).
    Per-partition partials [128, 1] land in accumulator tiles, DMA'd out;
    the final 128-way sums + dice/BCE math run on host in float64.
  - This walrus build rejects instructions carrying more than one sync
    wait; _split_excess_waits() moves surplus waits onto same-engine NoOps
    inserted just before (identical semantics on in-order sequencers).
"""

import sys

import numpy as np

for _p in ("/opt/trn_rl_repo",):
    if _p not in sys.path:
        sys.path.insert(0, _p)

import concourse.bass as bass
import concourse.tile as tile
from concourse import mybir
from concourse.bass_utils import run_bass_kernel_spmd

B, C, H, W = 64, 1, 512, 512
N_CORES = 8
N = C * H * W  # elements per sample = 262144
P = 128  # SBUF partitions
F = N // P  # free dim per sample tile = 2048

_F32 = mybir.dt.float32
_F16 = mybir.dt.float16

_split_ctr = [0]


def _split_excess_waits(nc: bass.Bass, max_waits: int = 1) -> bass.Bass:
    """Move surplus sync waits onto same-engine NoOps (walrus allows only
    one wait per instruction in this build)."""
    for bb in nc.main_func.blocks:
        insts = bb.instructions
        new = []
        changed = False
        for ins in insts:
            si = getattr(ins, "sync_info", None)
            waits = list(si.on_wait) if (si is not None and si.on_wait) else []
            if len(waits) > max_waits:
                keep = waits[-max_waits:]
                extra = waits[:-max_waits]
                for k in range(0, len(extra), max_waits):
                    chunk = extra[k : k + max_waits]
                    _split_ctr[0] += 1
                    new.append(
                        mybir.InstNoOp(
                            name=f"ant_wait_split_{_split_ctr[0]}",
                            engine=ins.engine,
                            ins=[],
                            outs=[],
                            sync_info=mybir.SyncInfo(on_wait=chunk, on_update=[]),
                        )
                    )
                ins.sync_info = mybir.SyncInfo(
                    on_wait=keep, on_update=list(si.on_update)
                )
                changed = True
            new.append(ins)
        if changed:
            insts[:] = new
    return nc


def _build_nc(K1: int, K0: int) -> bass.Bass:
    """Per-core program: K1 full samples (pg/pp/gg), K0 sig-only (psum)."""
    nc = bass.Bass()
    AF = mybir.ActivationFunctionType
    OP = mybir.AluOpType

    seg_full = msk_full = seg_sig = None
    if K1 > 0:
        seg_full = nc.declare_dram_parameter("seg_full", [K1, P, F], _F16, False)[:]
        msk_full = nc.declare_dram_parameter("msk_full", [K1, P, F], _F16, False)[:]
    if K0 > 0:
        seg_sig = nc.declare_dram_parameter("seg_sig", [K0, P, F], _F16, False)[:]
    # res_v[:, j] = pg_j ; res_v[:, K1+j] = pp_j ; res_v[:, 2K1+j] = gg_j
    res_v = (
        nc.declare_dram_parameter("res_v", [P, 3 * K1], _F32, True) if K1 else None
    )
    # res_a[:, j] = psum_j (sig-only samples)
    res_a = nc.declare_dram_parameter("res_a", [P, K0], _F32, True) if K0 else None

    with tile.TileContext(nc) as tc:
        with (
            tc.tile_pool(name="seg_p", bufs=3) as seg_p,
            tc.tile_pool(name="msk_p", bufs=3) as msk_p,
            tc.tile_pool(name="sig_p", bufs=2) as sig_p,
            tc.tile_pool(name="dmp_p", bufs=2) as dmp_p,
            tc.tile_pool(name="acc_p", bufs=1) as acc_p,
        ):
            acc_v = acc_p.tile([P, 3 * K1], _F32, name="acc_v") if K1 else None
            acc_a = acc_p.tile([P, K0], _F32, name="acc_a") if K0 else None
            # interleave full / sig-only samples for smoother pipelining
            order = []
            for j in range(max(K1, K0)):
                if j < K1:
                    order.append(("full", j))
                if j < K0:
                    order.append(("sig", j))
            for kind, j in order:
                if kind == "full":
                    s = seg_p.tile([P, F], _F16, tag="s")
                    m = msk_p.tile([P, F], _F16, tag="m")
                    nc.sync.dma_start(out=s, in_=seg_full[j])
                    nc.sync.dma_start(out=m, in_=msk_full[j])
                    g = sig_p.tile([P, F], _F16, tag="g")
                    nc.scalar.activation(g, s, AF.Sigmoid)
                    d0 = dmp_p.tile([P, F], _F16, tag="d0")
                    d1 = dmp_p.tile([P, F], _F16, tag="d1")
                    d2 = dmp_p.tile([P, F], _F16, tag="d2")
                    # pg = sum(g*m)
                    nc.vector.scalar_tensor_tensor(
                        out=d0, in0=g, scalar=1.0, in1=m,
                        op0=OP.mult, op1=OP.mult,
                        accum_out=acc_v[:, j : j + 1],
                    )
                    # pp = sum(g*g)
                    nc.vector.scalar_tensor_tensor(
                        out=d1, in0=g, scalar=1.0, in1=g,
                        op0=OP.mult, op1=OP.mult,
                        accum_out=acc_v[:, K1 + j : K1 + j + 1],
                    )
                    # gg = sum(m) (single-src tensor_scalar, 4x-mode candidate)
                    nc.vector.tensor_scalar(
                        out=d2, in0=m, scalar1=1.0, scalar2=0.0,
                        op0=OP.mult, op1=OP.add,
                        accum_out=acc_v[:, 2 * K1 + j : 2 * K1 + j + 1],
                    )
                else:
                    s = seg_p.tile([P, F], _F16, tag="s")
                    nc.sync.dma_start(out=s, in_=seg_sig[j])
                    g = sig_p.tile([P, F], _F16, tag="g")
                    # psum = sum(sigmoid(s))
                    nc.scalar.activation(
                        g, s, AF.Sigmoid, accum_out=acc_a[:, j : j + 1]
                    )
            if K1:
                nc.sync.dma_start(out=res_v[:], in_=acc_v)
            if K0:
                nc.sync.dma_start(out=res_a[:], in_=acc_a)
    return _split_excess_waits(nc)


_NC_CACHE: dict = {}


def _get_nc(K1: int, K0: int) -> bass.Bass:
    key = (K1, K0)
    if key not in _NC_CACHE:
        _NC_CACHE[key] = _build_nc(K1, K0)
    return _NC_CACHE[key]


def _device_sums(seg16, msk16, full_lists, sig_lists, K1, K0, **spmd_kwargs):
    """seg16/msk16: [B, P, F] fp16 views. full_lists/sig_lists: per-core
    sample-index lists (len <= K1/K0).  Returns dicts idx->float64 sums and
    the raw BassKernelResults."""
    in_maps = []
    for c in range(N_CORES):
        im = {}
        if K1:
            sf = np.zeros((K1, P, F), np.float16)
            mf = np.zeros((K1, P, F), np.float16)
            for j, idx in enumerate(full_lists[c]):
                sf[j] = seg16[idx]
                mf[j] = msk16[idx]
            im["seg_full"] = sf
            im["msk_full"] = mf
        if K0:
            ss = np.zeros((K0, P, F), np.float16)
            for j, idx in enumerate(sig_lists[c]):
                ss[j] = seg16[idx]
            im["seg_sig"] = ss
        in_maps.append(im)
    out = run_bass_kernel_spmd(
        _get_nc(K1, K0), in_maps, list(range(N_CORES)), **spmd_kwargs
    )
    pg, pp, gg, psum = {}, {}, {}, {}
    for c in range(N_CORES):
        if K1:
            rv = np.asarray(out.results[c]["res_v"], dtype=np.float64)
            for j, idx in enumerate(full_lists[c]):
                pg[idx] = rv[:, j].sum()
                pp[idx] = rv[:, K1 + j].sum()
                gg[idx] = rv[:, 2 * K1 + j].sum()
        if K0:
            ra = np.asarray(out.results[c]["res_a"], dtype=np.float64)
            for j, idx in enumerate(sig_lists[c]):
                psum[idx] = ra[:, j].sum()
    return pg, pp, gg, psum, out


def _plan(pc, lab):
    sel = pc >= 0.5
    L1 = [int(i) for i in np.nonzero(sel & (lab == 1.0))[0]]
    L0 = [int(i) for i in np.nonzero(sel & (lab != 1.0))[0]]
    full_lists = [L1[c::N_CORES] for c in range(N_CORES)]
    sig_lists = [L0[c::N_CORES] for c in range(N_CORES)]
    K1 = max((len(x) for x in full_lists), default=0)
    K0 = max((len(x) for x in sig_lists), default=0)
    return L1, L0, full_lists, sig_lists, K1, K0


def kernel(predict_cls, predict_seg, labels, masks):
    pc = np.asarray(predict_cls, dtype=np.float64)
    lab = np.asarray(labels).astype(np.float64)

    # classification BCE (mean reduction) -- O(B), host
    eps = 1e-7
    pc_c = np.clip(pc, eps, 1.0 - eps)
    cls_loss = -np.mean(lab * np.log(pc_c) + (1.0 - lab) * np.log(1.0 - pc_c))

    L1, L0, full_lists, sig_lists, K1, K0 = _plan(pc, lab)
    n = float(len(L1) + len(L0))
    if n == 0.0:
        return (np.float32(cls_loss), np.float32(1e-4))

    seg16 = np.asarray(predict_seg).reshape(B, P, F).astype(np.float16)
    msk16 = np.asarray(masks).reshape(B, P, F).astype(np.float16)
    pg, pp, gg, psum, _ = _device_sums(seg16, msk16, full_lists, sig_lists, K1, K0)

    dice_sum = 0.0
    for i in L1:
        dice_sum += (2.0 * pg[i] + 1e-5) / (pp[i] + gg[i] + 1e-5)
    for i in L0:
        dice_sum += 25.0 / (psum[i] + 25.0)
    seg_loss = (n - dice_sum) / max(n, 1.0)
    return (np.float32(cls_loss), np.float32(seg_loss))


# revision 6
# speedup vs baseline: 1.8347x; 1.0376x over previous
"""Trainium2 kernel for nn_ClsSegLoss (cls BCE + masked dice seg loss).

Strategy (data-parallel over batch, 8 NeuronCores):
  - cls BCE needs only predict_cls/labels (64 floats) -> host.
  - seg dice only involves samples with predict_cls >= 0.5 ("selected"):
      label==1 selected samples need pg=sum(sig*m), pp=sum(sig^2),
               gg=sum(m)  (mask binary -> == sum(m^2))
      label!=1 selected samples need only psum=sum(sig)
      unselected samples contribute nothing.
    predict_cls/labels are host-visible, so kernel() builds the exact work
    list per call and ships only the needed samples, fp16-cast (masks are
    exactly {0,1} -> fp16 lossless; fp16 logits add ~1e-5 rel err to the
    262144-element sums).
  - Per core: K1 "full" samples (seg+mask, ACT sigmoid -> DVE
    scalar_tensor_tensor pg/pp + tensor_scalar gg, all with fused fp32
    accum_out) and K0 "sig-only" samples (ACT sigmoid with accum_out).
    Per-partition partials [128, 1] land in accumulator tiles, DMA'd out;
    the final 128-way sums + dice/BCE math run on host in float64.
  - This walrus build rejects instructions carrying more than one sync
    wait; _split_excess_waits() moves surplus waits onto same-engine NoOps
    inserted just before (identical semantics on in-order sequencers).
"""

import sys

import numpy as np

for _p in ("/opt/trn_rl_repo",):
    if _p not in sys.path:
        sys.path.insert(0, _p)

import concourse.bass as bass
import concourse.tile as tile
from concourse import mybir
from concourse.bass_utils import run_bass_kernel_spmd

B, C, H, W = 64, 1, 512, 512
N_CORES = 8
N = C * H * W  # elements per sample = 262144
P = 128  # SBUF partitions
F = N // P  # free dim per sample tile = 2048

_F32 = mybir.dt.float32
_F16 = mybir.dt.float16

_split_ctr = [0]


def _split_excess_waits(nc: bass.Bass, max_waits: int = 1) -> bass.Bass:
    """Move surplus sync waits onto same-engine NoOps (walrus allows only
    one wait per instruction in this build)."""
    for bb in nc.main_func.blocks:
        insts = bb.instructions
        new = []
        changed = False
        for ins in insts:
            si = getattr(ins, "sync_info", None)
            waits = list(si.on_wait) if (si is not None and si.on_wait) else []
            if len(waits) > max_waits:
                keep = waits[-max_waits:]
                extra = waits[:-max_waits]
                for k in range(0, len(extra), max_waits):
                    chunk = extra[k : k + max_waits]
                    _split_ctr[0] += 1
                    new.append(
                        mybir.InstNoOp(
                            name=f"ant_wait_split_{_split_ctr[0]}",
                            engine=ins.engine,
                            ins=[],
                            outs=[],
                            sync_info=mybir.SyncInfo(on_wait=chunk, on_update=[]),
                        )
                    )
                ins.sync_info = mybir.SyncInfo(
                    on_wait=keep, on_update=list(si.on_update)
                )
                changed = True
            new.append(ins)
        if changed:
            insts[:] = new
    return nc


def _build_nc(K1: int, K0: int) -> bass.Bass:
    """Per-core program: K1 full samples (pg/pp/gg), K0 sig-only (psum)."""
    nc = bass.Bass()
    AF = mybir.ActivationFunctionType
    OP = mybir.AluOpType

    seg_full = msk_full = seg_sig = None
    if K1 > 0:
        seg_full = nc.declare_dram_parameter("seg_full", [K1, P, F], _F16, False)[:]
        msk_full = nc.declare_dram_parameter("msk_full", [K1, P, F], _F16, False)[:]
    if K0 > 0:
        seg_sig = nc.declare_dram_parameter("seg_sig", [K0, P, F], _F16, False)[:]
    # res_v[0, 3j:3j+3] = (pg_j, pp_j, gg_j), already fully reduced
    res_v = (
        nc.declare_dram_parameter("res_v", [1, 3 * K1], _F32, True) if K1 else None
    )
    # res_a[:, j] = per-partition psum partials (sig-only samples)
    res_a = nc.declare_dram_parameter("res_a", [P, K0], _F32, True) if K0 else None

    NCHUNK = 4  # matmul free-dim chunks (N=512 max per PSUM bank)
    CW = F // NCHUNK

    with tile.TileContext(nc) as tc:
        with (
            tc.tile_pool(name="seg_p", bufs=3) as seg_p,
            tc.tile_pool(name="msk_p", bufs=3) as msk_p,
            tc.tile_pool(name="sig_p", bufs=2) as sig_p,
            tc.tile_pool(name="dmp_p", bufs=2) as dmp_p,
            tc.tile_pool(name="acc_p", bufs=1) as acc_p,
            tc.tile_pool(name="ps_p", bufs=2, space="PSUM") as ps_p,
        ):
            acc_v = acc_p.tile([1, 3 * K1], _F32, name="acc_v") if K1 else None
            acc_a = acc_p.tile([P, K0], _F32, name="acc_a") if K0 else None
            ones = None
            if K1:
                ones = acc_p.tile([P, 1], _F16, name="ones")
                nc.vector.memset(ones, 1.0)
            # interleave full / sig-only samples for smoother pipelining
            order = []
            for j in range(max(K1, K0)):
                if j < K1:
                    order.append(("full", j))
                if j < K0:
                    order.append(("sig", j))
            for kind, j in order:
                if kind == "full":
                    s = seg_p.tile([P, F], _F16, tag="s")
                    m = msk_p.tile([P, F], _F16, tag="m")
                    nc.sync.dma_start(out=s, in_=seg_full[j])
                    nc.sync.dma_start(out=m, in_=msk_full[j])
                    g = sig_p.tile([P, F], _F16, tag="g")
                    nc.scalar.activation(g, s, AF.Sigmoid)
                    d0 = dmp_p.tile([P, F], _F16, tag="d0")
                    d1 = dmp_p.tile([P, F], _F16, tag="d1")
                    nc.vector.tensor_mul(d0, g, m)  # fp16 TT -> 2x mode
                    nc.vector.tensor_mul(d1, g, g)
                    # TensorE: ones^T @ src accumulates partition-sums into
                    # PSUM rows [1, CW]; 3 quantities x NCHUNK chunks.
                    ps = ps_p.tile([1, 3, CW], _F32, tag="ps")
                    for q, src in enumerate((d0, d1, m)):
                        srcv = src.rearrange("p (c w) -> p c w", c=NCHUNK)
                        for c in range(NCHUNK):
                            nc.tensor.matmul(
                                ps[:, q, :],
                                ones,
                                srcv[:, c, :],
                                start=(c == 0),
                                stop=(c == NCHUNK - 1),
                            )
                    # stage 2: [1, 3, CW] -> [1, 3] (pg_j, pp_j, gg_j)
                    nc.vector.tensor_reduce(
                        acc_v[:, 3 * j : 3 * j + 3],
                        ps,
                        axis=mybir.AxisListType.X,
                        op=OP.add,
                    )
                else:
                    s = seg_p.tile([P, F], _F16, tag="s")
                    nc.sync.dma_start(out=s, in_=seg_sig[j])
                    g = sig_p.tile([P, F], _F16, tag="g")
                    # psum = sum(sigmoid(s))
                    nc.scalar.activation(
                        g, s, AF.Sigmoid, accum_out=acc_a[:, j : j + 1]
                    )
            if K1:
                nc.sync.dma_start(out=res_v[:], in_=acc_v)
            if K0:
                nc.sync.dma_start(out=res_a[:], in_=acc_a)
    return _split_excess_waits(nc)


_NC_CACHE: dict = {}


def _get_nc(K1: int, K0: int) -> bass.Bass:
    key = (K1, K0)
    if key not in _NC_CACHE:
        _NC_CACHE[key] = _build_nc(K1, K0)
    return _NC_CACHE[key]


def _device_sums(seg16, msk16, full_lists, sig_lists, K1, K0, **spmd_kwargs):
    """seg16/msk16: [B, P, F] fp16 views. full_lists/sig_lists: per-core
    sample-index lists (len <= K1/K0).  Returns dicts idx->float64 sums and
    the raw BassKernelResults."""
    in_maps = []
    for c in range(N_CORES):
        im = {}
        if K1:
            sf = np.zeros((K1, P, F), np.float16)
            mf = np.zeros((K1, P, F), np.float16)
            for j, idx in enumerate(full_lists[c]):
                sf[j] = seg16[idx]
                mf[j] = msk16[idx]
            im["seg_full"] = sf
            im["msk_full"] = mf
        if K0:
            ss = np.zeros((K0, P, F), np.float16)
            for j, idx in enumerate(sig_lists[c]):
                ss[j] = seg16[idx]
            im["seg_sig"] = ss
        in_maps.append(im)
    out = run_bass_kernel_spmd(
        _get_nc(K1, K0), in_maps, list(range(N_CORES)), **spmd_kwargs
    )
    pg, pp, gg, psum = {}, {}, {}, {}
    for c in range(N_CORES):
        if K1:
            rv = np.asarray(out.results[c]["res_v"], dtype=np.float64)
            for j, idx in enumerate(full_lists[c]):
                pg[idx] = rv[0, 3 * j]
                pp[idx] = rv[0, 3 * j + 1]
                gg[idx] = rv[0, 3 * j + 2]
        if K0:
            ra = np.asarray(out.results[c]["res_a"], dtype=np.float64)
            for j, idx in enumerate(sig_lists[c]):
                psum[idx] = ra[:, j].sum()
    return pg, pp, gg, psum, out


def _plan(pc, lab):
    sel = pc >= 0.5
    L1 = [int(i) for i in np.nonzero(sel & (lab == 1.0))[0]]
    L0 = [int(i) for i in np.nonzero(sel & (lab != 1.0))[0]]
    full_lists = [L1[c::N_CORES] for c in range(N_CORES)]
    sig_lists = [L0[c::N_CORES] for c in range(N_CORES)]
    K1 = max((len(x) for x in full_lists), default=0)
    K0 = max((len(x) for x in sig_lists), default=0)
    return L1, L0, full_lists, sig_lists, K1, K0


def kernel(predict_cls, predict_seg, labels, masks):
    pc = np.asarray(predict_cls, dtype=np.float64)
    lab = np.asarray(labels).astype(np.float64)

    # classification BCE (mean reduction) -- O(B), host
    eps = 1e-7
    pc_c = np.clip(pc, eps, 1.0 - eps)
    cls_loss = -np.mean(lab * np.log(pc_c) + (1.0 - lab) * np.log(1.0 - pc_c))

    L1, L0, full_lists, sig_lists, K1, K0 = _plan(pc, lab)
    n = float(len(L1) + len(L0))
    if n == 0.0:
        return (np.float32(cls_loss), np.float32(1e-4))

    seg16 = np.asarray(predict_seg).reshape(B, P, F).astype(np.float16)
    msk16 = np.asarray(masks).reshape(B, P, F).astype(np.float16)
    pg, pp, gg, psum, _ = _device_sums(seg16, msk16, full_lists, sig_lists, K1, K0)

    dice_sum = 0.0
    for i in L1:
        dice_sum += (2.0 * pg[i] + 1e-5) / (pp[i] + gg[i] + 1e-5)
    for i in L0:
        dice_sum += 25.0 / (psum[i] + 25.0)
    seg_loss = (n - dice_sum) / max(n, 1.0)
    return (np.float32(cls_loss), np.float32(seg_loss))


# revision 11
# speedup vs baseline: 1.8666x; 1.0174x over previous
"""Trainium2 kernel for nn_ClsSegLoss (cls BCE + masked dice seg loss).

Strategy (data-parallel over batch, 8 NeuronCores):
  - cls BCE needs only predict_cls/labels (64 floats) -> host.
  - seg dice only involves samples with predict_cls >= 0.5 ("selected"):
      label==1 selected samples need pg=sum(sig*m), pp=sum(sig^2),
               gg=sum(m)  (mask binary -> == sum(m^2))
      label!=1 selected samples need only psum=sum(sig)
      unselected samples contribute nothing.
    predict_cls/labels are host-visible, so kernel() builds the exact work
    list per call and ships only the needed samples, fp16-cast (masks are
    exactly {0,1} -> fp16 lossless; fp16 logits add ~1e-5 rel err to the
    262144-element sums).
  - Per core: K1 "full" samples (seg+mask, ACT sigmoid -> DVE
    scalar_tensor_tensor pg/pp + tensor_scalar gg, all with fused fp32
    accum_out) and K0 "sig-only" samples (ACT sigmoid with accum_out).
    Per-partition partials [128, 1] land in accumulator tiles, DMA'd out;
    the final 128-way sums + dice/BCE math run on host in float64.
  - This walrus build rejects instructions carrying more than one sync
    wait; _split_excess_waits() moves surplus waits onto same-engine NoOps
    inserted just before (identical semantics on in-order sequencers).
"""

import sys

import numpy as np

for _p in ("/opt/trn_rl_repo",):
    if _p not in sys.path:
        sys.path.insert(0, _p)

import concourse.bass as bass
import concourse.tile as tile
from concourse import mybir
from concourse.bass_utils import run_bass_kernel_spmd

B, C, H, W = 64, 1, 512, 512
N_CORES = 8
N = C * H * W  # elements per sample = 262144
P = 128  # SBUF partitions
F = N // P  # free dim per sample tile = 2048

_F32 = mybir.dt.float32
_F16 = mybir.dt.float16

_split_ctr = [0]


def _split_excess_waits(nc: bass.Bass, max_waits: int = 1) -> bass.Bass:
    """Move surplus sync waits onto same-engine NoOps (walrus allows only
    one wait per instruction in this build)."""
    for bb in nc.main_func.blocks:
        insts = bb.instructions
        new = []
        changed = False
        for ins in insts:
            si = getattr(ins, "sync_info", None)
            waits = list(si.on_wait) if (si is not None and si.on_wait) else []
            if len(waits) > max_waits:
                keep = waits[-max_waits:]
                extra = waits[:-max_waits]
                for k in range(0, len(extra), max_waits):
                    chunk = extra[k : k + max_waits]
                    _split_ctr[0] += 1
                    new.append(
                        mybir.InstNoOp(
                            name=f"ant_wait_split_{_split_ctr[0]}",
                            engine=ins.engine,
                            ins=[],
                            outs=[],
                            sync_info=mybir.SyncInfo(on_wait=chunk, on_update=[]),
                        )
                    )
                ins.sync_info = mybir.SyncInfo(
                    on_wait=keep, on_update=list(si.on_update)
                )
                changed = True
            new.append(ins)
        if changed:
            insts[:] = new
    return nc


def _build_nc(K1: int, K0: int) -> bass.Bass:
    """Per-core program: K1 full samples (pg/pp/gg), K0 sig-only (psum)."""
    nc = bass.Bass()
    AF = mybir.ActivationFunctionType
    OP = mybir.AluOpType

    seg_full = msk_full = seg_sig = None
    if K1 > 0:
        seg_full = nc.declare_dram_parameter("seg_full", [K1, P, F], _F16, False)[:]
        msk_full = nc.declare_dram_parameter("msk_full", [K1, P, F], _F16, False)[:]
    if K0 > 0:
        seg_sig = nc.declare_dram_parameter("seg_sig", [K0, P, F], _F16, False)[:]
    # res_v[0, 2j:2j+2] = (pp_j, gg_j), already fully reduced
    res_v = (
        nc.declare_dram_parameter("res_v", [1, 2 * K1], _F32, True) if K1 else None
    )
    # res_pg[:, j] = per-partition pg partials (STT accum)
    res_pg = nc.declare_dram_parameter("res_pg", [P, K1], _F32, True) if K1 else None
    # res_a[:, j] = per-partition psum partials (sig-only samples)
    res_a = nc.declare_dram_parameter("res_a", [P, K0], _F32, True) if K0 else None

    NCHUNK = 4  # matmul free-dim chunks (N=512 max per PSUM bank)
    CW = F // NCHUNK

    with tile.TileContext(nc) as tc:
        with (
            tc.tile_pool(name="seg_p", bufs=3) as seg_p,
            tc.tile_pool(name="msk_p", bufs=3) as msk_p,
            tc.tile_pool(name="sig_p", bufs=2) as sig_p,
            tc.tile_pool(name="dmp_p", bufs=2) as dmp_p,
            tc.tile_pool(name="acc_p", bufs=1) as acc_p,
            tc.tile_pool(name="ps_p", bufs=2, space="PSUM") as ps_p,
        ):
            acc_v = acc_p.tile([1, 2 * K1], _F32, name="acc_v") if K1 else None
            acc_pg = acc_p.tile([P, K1], _F32, name="acc_pg") if K1 else None
            acc_a = acc_p.tile([P, K0], _F32, name="acc_a") if K0 else None
            ones = None
            if K1:
                ones = acc_p.tile([P, 1], _F16, name="ones")
                nc.vector.memset(ones, 1.0)
            # interleave full / sig-only samples for smoother pipelining
            order = []
            for j in range(max(K1, K0)):
                if j < K1:
                    order.append(("full", j))
                if j < K0:
                    order.append(("sig", j))
            for kind, j in order:
                if kind == "full":
                    s = seg_p.tile([P, F], _F16, tag="s")
                    m = msk_p.tile([P, F], _F16, tag="m")
                    nc.sync.dma_start(out=s, in_=seg_full[j])
                    nc.sync.dma_start(out=m, in_=msk_full[j])
                    g = sig_p.tile([P, F], _F16, tag="g")
                    nc.scalar.activation(g, s, AF.Sigmoid)
                    # pg = sum(g*m): one fused DVE pass with fp32 accum
                    d0 = dmp_p.tile([P, F], _F16, tag="d0")
                    nc.vector.scalar_tensor_tensor(
                        out=d0, in0=g, scalar=1.0, in1=m,
                        op0=OP.mult, op1=OP.mult,
                        accum_out=acc_pg[:, j : j + 1],
                    )
                    # pp: g*g product on DVE (fp16 TT -> 2x), reduced on PE
                    d1 = dmp_p.tile([P, F], _F16, tag="d1")
                    nc.vector.tensor_mul(d1, g, g)
                    # TensorE: ones^T @ src accumulates partition-sums into
                    # PSUM rows [1, CW]; (pp, gg) x NCHUNK chunks.
                    ps = ps_p.tile([1, 2, CW], _F32, tag="ps")
                    for q, src in enumerate((d1, m)):
                        srcv = src.rearrange("p (c w) -> p c w", c=NCHUNK)
                        for c in range(NCHUNK):
                            nc.tensor.matmul(
                                ps[:, q, :],
                                ones,
                                srcv[:, c, :],
                                start=(c == 0),
                                stop=(c == NCHUNK - 1),
                            )
                    # stage 2: [1, 2, CW] -> [1, 2] (pp_j, gg_j)
                    nc.vector.tensor_reduce(
                        acc_v[:, 2 * j : 2 * j + 2],
                        ps,
                        axis=mybir.AxisListType.X,
                        op=OP.add,
                    )
                else:
                    s = seg_p.tile([P, F], _F16, tag="s")
                    nc.sync.dma_start(out=s, in_=seg_sig[j])
                    g = sig_p.tile([P, F], _F16, tag="g")
                    # psum = sum(sigmoid(s))
                    nc.scalar.activation(
                        g, s, AF.Sigmoid, accum_out=acc_a[:, j : j + 1]
                    )
            if K1:
                nc.sync.dma_start(out=res_v[:], in_=acc_v)
                nc.sync.dma_start(out=res_pg[:], in_=acc_pg)
            if K0:
                nc.sync.dma_start(out=res_a[:], in_=acc_a)
    return _split_excess_waits(nc)


_NC_CACHE: dict = {}


def _get_nc(K1: int, K0: int) -> bass.Bass:
    key = (K1, K0)
    if key not in _NC_CACHE:
        _NC_CACHE[key] = _build_nc(K1, K0)
    return _NC_CACHE[key]


def _device_sums(seg16, msk16, full_lists, sig_lists, K1, K0, **spmd_kwargs):
    """seg16/msk16: [B, P, F] fp16 views. full_lists/sig_lists: per-core
    sample-index lists (len <= K1/K0).  Returns dicts idx->float64 sums and
    the raw BassKernelResults."""
    in_maps = []
    for c in range(N_CORES):
        im = {}
        if K1:
            sf = np.zeros((K1, P, F), np.float16)
            mf = np.zeros((K1, P, F), np.float16)
            for j, idx in enumerate(full_lists[c]):
                sf[j] = seg16[idx]
                mf[j] = msk16[idx]
            im["seg_full"] = sf
            im["msk_full"] = mf
        if K0:
            ss = np.zeros((K0, P, F), np.float16)
            for j, idx in enumerate(sig_lists[c]):
                ss[j] = seg16[idx]
            im["seg_sig"] = ss
        in_maps.append(im)
    out = run_bass_kernel_spmd(
        _get_nc(K1, K0), in_maps, list(range(N_CORES)), **spmd_kwargs
    )
    pg, pp, gg, psum = {}, {}, {}, {}
    for c in range(N_CORES):
        if K1:
            rv = np.asarray(out.results[c]["res_v"], dtype=np.float64)
            rpg = np.asarray(out.results[c]["res_pg"], dtype=np.float64)
            for j, idx in enumerate(full_lists[c]):
                pg[idx] = rpg[:, j].sum()
                pp[idx] = rv[0, 2 * j]
                gg[idx] = rv[0, 2 * j + 1]
        if K0:
            ra = np.asarray(out.results[c]["res_a"], dtype=np.float64)
            for j, idx in enumerate(sig_lists[c]):
                psum[idx] = ra[:, j].sum()
    return pg, pp, gg, psum, out


def _plan(pc, lab):
    sel = pc >= 0.5
    L1 = [int(i) for i in np.nonzero(sel & (lab == 1.0))[0]]
    L0 = [int(i) for i in np.nonzero(sel & (lab != 1.0))[0]]
    full_lists = [L1[c::N_CORES] for c in range(N_CORES)]
    sig_lists = [L0[c::N_CORES] for c in range(N_CORES)]
    K1 = max((len(x) for x in full_lists), default=0)
    K0 = max((len(x) for x in sig_lists), default=0)
    return L1, L0, full_lists, sig_lists, K1, K0


def kernel(predict_cls, predict_seg, labels, masks):
    pc = np.asarray(predict_cls, dtype=np.float64)
    lab = np.asarray(labels).astype(np.float64)

    # classification BCE (mean reduction) -- O(B), host
    eps = 1e-7
    pc_c = np.clip(pc, eps, 1.0 - eps)
    cls_loss = -np.mean(lab * np.log(pc_c) + (1.0 - lab) * np.log(1.0 - pc_c))

    L1, L0, full_lists, sig_lists, K1, K0 = _plan(pc, lab)
    n = float(len(L1) + len(L0))
    if n == 0.0:
        return (np.float32(cls_loss), np.float32(1e-4))

    seg16 = np.asarray(predict_seg).reshape(B, P, F).astype(np.float16)
    msk16 = np.asarray(masks).reshape(B, P, F).astype(np.float16)
    pg, pp, gg, psum, _ = _device_sums(seg16, msk16, full_lists, sig_lists, K1, K0)

    dice_sum = 0.0
    for i in L1:
        dice_sum += (2.0 * pg[i] + 1e-5) / (pp[i] + gg[i] + 1e-5)
    for i in L0:
        dice_sum += 25.0 / (psum[i] + 25.0)
    seg_loss = (n - dice_sum) / max(n, 1.0)
    return (np.float32(cls_loss), np.float32(seg_loss))
